# revision 7
# baseline (speedup 1.0000x reference)
"""Trainium2 Bass kernel for a 2-layer GAT + global-max-pool + MLP (GATNet2).

Strategy (8 NeuronCores, data-parallel over destination nodes / graphs):
  - Nodes are sharded across cores at graph boundaries (batch is sorted).
  - 3 SPMD launches:
      P1 "embed":  per-core  h1 = x @ [W1|wsrc1|wdst1]          -> table1, a1
      P2 "mid":    per-core  layer-1 edge phase (gather h1[src] via
                   dma_gather, attention-softmax-weighted segment-sum via
                   one-hot matmuls into PSUM), +bias, ELU, then @ W2c
                                                                 -> table2, a2
      P3 "final":  layer-2 edge phase, +bias, ELU, graph max-pool (PE
                   transpose + free-dim reduce over graph-aligned slot
                   blocks), 2-layer MLP                          -> logits
  - Host work between launches is indexing/assembly only: shard edges by
    dst, build int16 gather indices (lo/hi table split for the 32768 int16
    limit), gather per-edge attention pre-activations u = a_src[s]+a_dst[d]
    from the small device-computed a-tables.

Numerics: segment-softmax is computed without the segment-max subtraction
(logits are O(10), exp is safe in fp32); normalization is applied to the
accumulated sums instead of per-edge alphas. Both are mathematically
identical to the reference up to fp32 rounding.
"""
import os
import sys
import numpy as np

for _p in ("/opt/trn_rl_repo", "/root/.axon_site/_ro/trn_rl_repo"):
    if os.path.isdir(_p) and _p not in sys.path:
        sys.path.insert(0, _p)

# ---------------- problem constants (hardcoded per contest rules) ----------
NN = 50000        # nodes
EE = 400000       # edges (before self loops)
FIN = 256         # input features
HEADS = 4
CDIM = 64
HC = HEADS * CDIM  # 256
GRAPHS = 128
NCORE = 8
P = 128
LO = 32768        # int16 index limit -> lo/hi table split
NEG = 0.2         # leaky relu slope
ROW = 256         # gather row width (fp32 -> 1024B, %256==0)
AW = ROW + 8      # combined weight output width (h | a_src | a_dst)
PADLOC = 999.0    # dst-slot value for padding edges (matches no one-hot col)
MASKVAL = -1e30


# ---------------- host-side grid / sharding ---------------------------------
class Grid:
    pass


def _core_bounds(batch):
    """Graph-aligned node ranges per core + graph ranges."""
    gcounts = np.bincount(batch, minlength=GRAPHS)
    cum = np.concatenate([[0], np.cumsum(gcounts)])  # [G+1] node offset per graph
    gb = [0]
    for c in range(1, NCORE):
        target = c * NN / NCORE
        gi = int(np.argmin(np.abs(cum - target)))
        gi = min(max(gi, gb[-1] + 1), GRAPHS - (NCORE - c))
        gb.append(gi)
    gb.append(GRAPHS)
    nb = [int(cum[g]) for g in gb]
    return np.array(nb), np.array(gb), gcounts, cum


def _build_edge_grid(s, d, nb, slot_of_local, nslots):
    """Static (block, half) tile schedule shared by all cores.

    s, d: global src/dst of all edges (self loops included)
    slot_of_local[c]: maps core-local node id -> slot id (identity for P2)
    nslots[c]: number of slots on core c
    Returns Grid with per-core packed arrays.
    """
    g = Grid()
    B = max(-(-int(n) // P) for n in nslots)
    g.B = B
    percore = []
    cnt = np.zeros((NCORE, B, 2), np.int64)
    for c in range(NCORE):
        m = (d >= nb[c]) & (d < nb[c + 1])
        es = s[m]
        ed = d[m]
        slot = slot_of_local[c][ed - nb[c]]
        blk = slot // P
        loc = slot % P
        half = (es >= LO).astype(np.int64)
        order = np.lexsort((half, blk))
        es, ed, blk, loc, half = es[order], ed[order], blk[order], loc[order], half[order]
        np.add.at(cnt[c], (blk, half), 1)
        percore.append((es, ed, blk, loc, half))
    K = -(-cnt.max(axis=0) // P)          # [B, 2] tiles per (block, half)
    g.K = K
    groups = []
    t0 = 0
    for b in range(B):
        for h in (0, 1):
            if K[b, h] > 0:
                groups.append((b, h, t0, int(K[b, h])))
                t0 += int(K[b, h])
    g.groups = groups
    g.T = t0

    g.idx16 = []
    g.dstloc = []
    g.esg = []
    g.edg = []
    for c in range(NCORE):
        es, ed, blk, loc, half = percore[c]
        # offsets of each (b, h) run in the sorted arrays
        off = np.zeros((B, 2), np.int64)
        run = np.cumsum(cnt[c].ravel())
        off.ravel()[1:] = run[:-1]
        idx16 = np.zeros((16, g.T * 8), np.int16)
        dstloc = np.full((P, g.T), PADLOC, np.float32)
        esg = np.zeros((P, g.T), np.int64)
        edg = np.zeros((P, g.T), np.int64)
        for (b, h, t0, Kbh) in groups:
            L = int(cnt[c, b, h])
            if L == 0:
                continue
            o = int(off[b, h])
            k = np.arange(L)
            tt = t0 + k // P
            pp = k % P
            dstloc[pp, tt] = loc[o:o + L]
            esg[pp, tt] = es[o:o + L]
            edg[pp, tt] = ed[o:o + L]
            idx16[k % 16, t0 * 8 + k // 16] = (es[o:o + L] - (LO if h else 0)).astype(np.int16)
        g.idx16.append(np.tile(idx16, (8, 1)))
        g.dstloc.append(dstloc)
        g.esg.append(esg)
        g.edg.append(edg)
    return g


def _prep_structure(edge_index, batch):
    """All input-data-dependent static structure (computed once)."""
    st = Grid()
    s0 = np.asarray(edge_index[0], np.int64)
    d0 = np.asarray(edge_index[1], np.int64)
    loop = np.arange(NN, dtype=np.int64)
    s = np.concatenate([s0, loop])
    d = np.concatenate([d0, loop])
    nb, gb, gcounts, cum = _core_bounds(np.asarray(batch, np.int64))
    st.nb, st.gb, st.gcounts, st.cum = nb, gb, gcounts, cum

    # ---- P2 grid: slots = local node ids
    ident = [np.arange(nb[c + 1] - nb[c], dtype=np.int64) for c in range(NCORE)]
    nslots = [nb[c + 1] - nb[c] for c in range(NCORE)]
    st.g2 = _build_edge_grid(s, d, nb, ident, nslots)

    # ---- P3 grid: graph-padded slots
    st.Gmax = max(int(gb[c + 1] - gb[c]) for c in range(NCORE))
    assert st.Gmax <= P
    # blocks per graph-position i: max over cores of ceil(nodes(c,i)/P)
    Pi = np.zeros(st.Gmax, np.int64)
    for c in range(NCORE):
        for i in range(gb[c + 1] - gb[c]):
            Pi[i] = max(Pi[i], -(-int(gcounts[gb[c] + i]) // P))
    Pi = np.maximum(Pi, 1)
    st.Pi = Pi
    blk_base = np.concatenate([[0], np.cumsum(Pi)])
    st.B3 = int(blk_base[-1])
    st.blk2graph = np.concatenate([np.full(int(Pi[i]), i, np.int64)
                                   for i in range(st.Gmax)])
    slotmaps = []
    poolmasks = []
    for c in range(NCORE):
        ncg = int(gb[c + 1] - gb[c])
        smap = np.zeros(nb[c + 1] - nb[c], np.int64)
        pmask = np.full((P, st.B3), MASKVAL, np.float32)
        for i in range(ncg):
            gs = int(cum[gb[c] + i]) - nb[c]       # local node start
            ge = int(cum[gb[c] + i + 1]) - nb[c]
            base = int(blk_base[i]) * P
            smap[gs:ge] = base + np.arange(ge - gs)
            for bb in range(int(Pi[i])):
                v0 = bb * P
                v1 = min(ge - gs, (bb + 1) * P)
                if v1 > v0:
                    pmask[: v1 - v0, int(blk_base[i]) + bb] = 0.0
        slotmaps.append(smap)
        poolmasks.append(pmask)
    st.poolmasks = poolmasks
    st.g3 = _build_edge_grid(s, d, nb, slotmaps, [st.B3 * P] * NCORE)
    return st


def _combine_w(W, att_src, att_dst):
    W = np.asarray(W, np.float64)
    ws = (W.reshape(FIN if W.shape[0] == FIN else HC, HEADS, CDIM)
          * np.asarray(att_src, np.float64)[None]).sum(-1)
    wd = (W.reshape(-1, HEADS, CDIM)
          * np.asarray(att_dst, np.float64)[None]).sum(-1)
    return np.concatenate([W, ws, wd], axis=1).astype(np.float32)  # [K, 264]


def _build_u(a_full, esg, edg):
    """Per-edge attention pre-activation u = a_src[s] + a_dst[d], packed."""
    a_src = a_full[:, 0:4]
    a_dst = a_full[:, 4:8]
    u = a_src[esg] + a_dst[edg]          # [P, T, 4]
    return np.ascontiguousarray(u.reshape(P, -1), dtype=np.float32)


# ---------------- numpy emulation of the device programs ---------------------
def _np_edge_phase(grid, table, u, c):
    """Emulate P2/P3 edge phase for core c -> per-block [P, 260] results."""
    ex = np.exp(np.where(u > 0, u, NEG * u)).reshape(P, grid.T, 4)
    dstloc = grid.dstloc[c]
    esg = grid.esg[c]
    out = np.zeros((grid.B, P, ROW + 4), np.float32)
    for (b, h, t0, K) in grid.groups:
        for k in range(K):
            t = t0 + k
            onehot = (dstloc[:, t][:, None] == np.arange(P)[None, :]).astype(np.float32)
            gath = table[esg[:, t]]                       # [P, 256]
            msg = np.empty((P, ROW + 4), np.float32)
            msg[:, :ROW] = (gath.reshape(P, 4, 64) * ex[:, t, :, None]).reshape(P, ROW)
            msg[:, ROW:] = ex[:, t]
            out[b] += onehot.T @ msg
    return out


def _np_norm_elu(blk, bias):
    den = blk[:, ROW:] + 1e-16
    go = (blk[:, :ROW].reshape(P, 4, 64) / den[:, :, None]).reshape(P, ROW)
    z = go + bias[None, :]
    return np.where(z > 0, z, np.exp(np.minimum(z, 0)) - 1).astype(np.float32)


def _run_numpy(st, inputs):
    x = np.asarray(inputs['x'], np.float32)
    w1c = _combine_w(inputs['W1'], inputs['att_src1'], inputs['att_dst1'])
    w2c = _combine_w(inputs['W2'], inputs['att_src2'], inputs['att_dst2'])
    b1 = np.asarray(inputs['b1'], np.float32)
    b2 = np.asarray(inputs['b2'], np.float32)
    # P1
    h1full = x @ w1c                       # [N, 264]
    table1 = np.ascontiguousarray(h1full[:, :ROW])
    a1 = h1full[:, ROW:]
    # P2
    table2 = np.zeros((NN, ROW), np.float32)
    a2 = np.zeros((NN, 8), np.float32)
    for c in range(NCORE):
        u = _build_u(a1, st.g2.esg[c], st.g2.edg[c])
        blks = _np_edge_phase(st.g2, table1, u, c)
        n0, n1 = st.nb[c], st.nb[c + 1]
        rows = []
        for b in range(st.g2.B):
            rows.append(_np_norm_elu(blks[b], b1))
        helu = np.concatenate(rows, 0)[: n1 - n0]
        h2 = helu @ w2c
        table2[n0:n1] = h2[:, :ROW]
        a2[n0:n1] = h2[:, ROW:]
    # P3
    z = np.zeros((GRAPHS, 2), np.float32)
    lin1_w = np.asarray(inputs['lin1_w'], np.float32)
    lin1_b = np.asarray(inputs['lin1_b'], np.float32)
    lin2_w = np.asarray(inputs['lin2_w'], np.float32)
    lin2_b = np.asarray(inputs['lin2_b'], np.float32)
    for c in range(NCORE):
        u = _build_u(a2, st.g3.esg[c], st.g3.edg[c])
        blks = _np_edge_phase(st.g3, table2, u, c)
        acc = np.full((st.Gmax, ROW), MASKVAL, np.float32)
        for b in range(st.B3):
            helu = _np_norm_elu(blks[b], b2)
            hm = helu + st.poolmasks[c][:, b][:, None]
            i = st.blk2graph[b]
            acc[i] = np.maximum(acc[i], hm.max(axis=0))
        ncg = st.gb[c + 1] - st.gb[c]
        z1 = acc[:ncg] @ lin1_w + lin1_b
        z2 = z1 @ lin2_w + lin2_b
        z[st.gb[c]:st.gb[c + 1]] = z2
    return z


# ---------------- Trainium programs ------------------------------------------
_DEV_CACHE = {}


def _get_bass_modules():
    import concourse.bass as bass
    import concourse.mybir as mybir
    import concourse.tile as tile
    import concourse.bacc as bacc
    from concourse import bass_utils
    from concourse.library_config import mlp
    from concourse.masks import make_identity
    return bass, mybir, tile, bacc, bass_utils, mlp, make_identity


def _build_p1(B):
    bass, mybir, tile, bacc, bass_utils, mlp, make_identity = _get_bass_modules()
    from contextlib import ExitStack
    F32 = mybir.dt.float32
    nc = bacc.Bacc("TRN2", target_bir_lowering=False, debug=False, num_devices=NCORE)
    xT = nc.dram_tensor("xT", [FIN, B * P], F32, kind="ExternalInput").ap()
    w1c = nc.dram_tensor("w1c", [FIN, AW], F32, kind="ExternalInput").ap()
    out = nc.dram_tensor("out", [B * P, AW], F32, kind="ExternalOutput").ap()
    with tile.TileContext(nc) as tc, ExitStack() as ctx:
        sb = ctx.enter_context(tc.tile_pool(name="sb", bufs=3))
        wp = ctx.enter_context(tc.tile_pool(name="wp", bufs=1))
        ps = ctx.enter_context(tc.tile_pool(name="ps", bufs=2, space="PSUM"))
        wt = []
        for j in range(2):
            w = wp.tile([P, AW], F32, tag=f"w{j}")
            nc.sync.dma_start(w[:], w1c[j * P:(j + 1) * P, :])
            wt.append(w)
        for b in range(B):
            pso = ps.tile([P, AW], F32, space="PSUM", tag="pso")
            for j in range(2):
                lh = sb.tile([P, P], F32, tag="lh")
                nc.sync.dma_start(lh[:], xT[j * P:(j + 1) * P, b * P:(b + 1) * P])
                nc.tensor.matmul(out=pso[:], lhsT=lh[:], rhs=wt[j][:],
                                 start=(j == 0), stop=(j == 1))
            ot = sb.tile([P, AW], F32, tag="ot")
            nc.vector.tensor_copy(ot[:], pso[:])
            nc.sync.dma_start(out[b * P:(b + 1) * P, :], ot[:])
    nc.compile()
    return nc


def _build_edge_program(grid, st, final):
    bass, mybir, tile, bacc, bass_utils, mlp, make_identity = _get_bass_modules()
    from contextlib import ExitStack
    F32 = mybir.dt.float32
    I16 = mybir.dt.int16
    I32 = mybir.dt.int32
    AF = mybir.ActivationFunctionType
    OP = mybir.AluOpType
    T, B = grid.T, (st.B3 if final else grid.B)

    nc = bacc.Bacc("TRN2", target_bir_lowering=False, debug=False, num_devices=NCORE)
    table = nc.dram_tensor("table", [NN, ROW], F32, kind="ExternalInput").ap()
    idx16 = nc.dram_tensor("idx16", [P, T * 8], I16, kind="ExternalInput").ap()
    dstloc = nc.dram_tensor("dstloc", [P, T], F32, kind="ExternalInput").ap()
    uarr = nc.dram_tensor("uarr", [P, T * 4], F32, kind="ExternalInput").ap()
    biasb = nc.dram_tensor("biasb", [P, ROW], F32, kind="ExternalInput").ap()
    if final:
        pmask = nc.dram_tensor("pmask", [P, B], F32, kind="ExternalInput").ap()
        lin1w = nc.dram_tensor("lin1w", [HC, HC // 2], F32, kind="ExternalInput").ap()
        lin1bb = nc.dram_tensor("lin1bb", [P, HC // 2], F32, kind="ExternalInput").ap()
        lin2w = nc.dram_tensor("lin2w", [HC // 2, 2], F32, kind="ExternalInput").ap()
        lin2bb = nc.dram_tensor("lin2bb", [P, 2], F32, kind="ExternalInput").ap()
        out = nc.dram_tensor("out", [P, 2], F32, kind="ExternalOutput").ap()
    else:
        w2c = nc.dram_tensor("w2c", [HC, AW], F32, kind="ExternalInput").ap()
        out = nc.dram_tensor("out", [B * P, AW], F32, kind="ExternalOutput").ap()

    with tile.TileContext(nc) as tc, ExitStack() as ctx:
        cons = ctx.enter_context(tc.tile_pool(name="cons", bufs=1))
        meta = ctx.enter_context(tc.tile_pool(name="meta", bufs=1))
        gp = ctx.enter_context(tc.tile_pool(name="gp", bufs=3))
        mp = ctx.enter_context(tc.tile_pool(name="mp", bufs=4))
        ohp = ctx.enter_context(tc.tile_pool(name="ohp", bufs=4))
        np_ = ctx.enter_context(tc.tile_pool(name="np", bufs=2))
        ps = ctx.enter_context(tc.tile_pool(name="ps", bufs=2, space="PSUM"))
        pst = ctx.enter_context(tc.tile_pool(name="pst", bufs=2, space="PSUM"))
        psh = ctx.enter_context(tc.tile_pool(name="psh", bufs=1, space="PSUM"))

        nc.gpsimd.load_library(mlp)

        # constants
        iota_i = cons.tile([P, P], I32)
        nc.gpsimd.iota(iota_i[:], pattern=[[1, P]], base=0, channel_multiplier=0)
        iota_f = cons.tile([P, P], F32)
        nc.vector.tensor_copy(iota_f[:], iota_i[:])
        ident = cons.tile([P, P], F32)
        make_identity(nc, ident[:])
        bias_t = cons.tile([P, ROW], F32)
        nc.sync.dma_start(bias_t[:], biasb[:, :])

        # metadata loads
        idx_t = meta.tile([P, T * 8], I16)
        nc.sync.dma_start(idx_t[:], idx16[:, :])
        dl_t = meta.tile([P, T], F32)
        nc.sync.dma_start(dl_t[:], dstloc[:, :])
        ex_t = meta.tile([P, T * 4], F32)

        # ex pre-pass: ex = exp(leaky_relu(u)); lrelu(u) = u + 0.8*relu(-u)
        CH = 2048
        u_t = meta.tile([P, T * 4], F32)
        nc.sync.dma_start(u_t[:], uarr[:, :])
        for o in range(0, T * 4, CH):
            e = min(o + CH, T * 4)
            rn = mp.tile([P, CH], F32, tag="rn")
            nc.scalar.activation(rn[:, :e - o], u_t[:, o:e], AF.Relu, scale=-1.0)
            nc.vector.scalar_tensor_tensor(ex_t[:, o:e], rn[:, :e - o], 0.8,
                                           u_t[:, o:e], op0=OP.mult, op1=OP.add)
            nc.scalar.activation(ex_t[:, o:e], ex_t[:, o:e], AF.Exp)

        if final:
            pm_t = meta.tile([P, B], F32)
            nc.sync.dma_start(pm_t[:], pmask[:, :])
            acc = cons.tile([P, 2 * st.Gmax], F32)
            nc.vector.memset(acc[:], MASKVAL)
            l1w = []
            for j in range(2):
                w = cons.tile([P, HC // 2], F32, tag=f"l1w{j}")
                nc.sync.dma_start(w[:], lin1w[j * P:(j + 1) * P, :])
                l1w.append(w)
            l1b = cons.tile([P, HC // 2], F32)
            nc.sync.dma_start(l1b[:], lin1bb[:, :])
            l2w = cons.tile([P, 2], F32)
            nc.sync.dma_start(l2w[:], lin2w[:, :])
            l2b = cons.tile([P, 2], F32)
            nc.sync.dma_start(l2b[:], lin2bb[:, :])
        else:
            w2t = []
            for j in range(2):
                w = cons.tile([P, AW], F32, tag=f"w2{j}")
                nc.sync.dma_start(w[:], w2c[j * P:(j + 1) * P, :])
                w2t.append(w)

        # group schedule per block
        by_block = [[] for _ in range(B)]
        for (b, h, t0, K) in grid.groups:
            by_block[b].append((h, t0, K))

        for b in range(B):
            glist = by_block[b]
            pso = ps.tile([P, ROW + 4], F32, space="PSUM", tag="pso")
            nt = sum(K for (_h, _t0, K) in glist)
            done = 0
            if nt == 0:
                continue
            for (h, t0, K) in glist:
                gb_t = gp.tile([P, K * ROW], F32, tag="gath")
                src_ap = table[:LO, :] if h == 0 else table[LO:, :]
                nc.gpsimd.dma_gather(
                    out_ap=gb_t[:].rearrange("p (k d) -> p k d", d=ROW),
                    in_ap=src_ap,
                    idxs_ap=idx_t[:, t0 * 8:(t0 + K) * 8],
                    num_idxs=K * P,
                    num_idxs_reg=K * P,
                    elem_size=ROW,
                )
                for k in range(K):
                    t = t0 + k
                    oh = ohp.tile([P, P], F32, tag="oh")
                    nc.vector.tensor_tensor(
                        out=oh[:], in0=dl_t[:, t:t + 1].to_broadcast([P, P]),
                        in1=iota_f[:], op=OP.is_equal)
                    msg = mp.tile([P, ROW + 4], F32, tag="msg")
                    nc.vector.tensor_tensor(
                        out=msg[:, :ROW].rearrange("p (h c) -> p h c", c=CDIM),
                        in0=gb_t[:, k * ROW:(k + 1) * ROW].rearrange(
                            "p (h c) -> p h c", c=CDIM),
                        in1=ex_t[:, t * 4:(t + 1) * 4].to_broadcast([P, 4, CDIM]),
                        op=OP.mult)
                    nc.vector.tensor_copy(msg[:, ROW:], ex_t[:, t * 4:(t + 1) * 4])
                    nc.tensor.matmul(out=pso[:], lhsT=oh[:], rhs=msg[:],
                                     start=(done == 0), stop=(done == nt - 1))
                    done += 1
            # normalize + bias + ELU
            den = np_.tile([P, 4], F32, tag="den")
            nc.vector.tensor_scalar_add(den[:], pso[:, ROW:], 1e-16)
            rec = np_.tile([P, 4], F32, tag="rec")
            nc.vector.reciprocal(rec[:], den[:])
            zb = np_.tile([P, ROW], F32, tag="zb")
            nc.vector.tensor_tensor(
                out=zb[:].rearrange("p (h c) -> p h c", c=CDIM),
                in0=pso[:, :ROW].rearrange("p (h c) -> p h c", c=CDIM),
                in1=rec[:].to_broadcast([P, 4, CDIM]), op=OP.mult)
            nc.vector.tensor_tensor(out=zb[:], in0=zb[:], in1=bias_t[:], op=OP.add)
            # elu: rp + exp(min(z,0)) - 1
            rn2 = np_.tile([P, ROW], F32, tag="rn2")
            nc.scalar.activation(rn2[:], zb[:], AF.Relu, scale=-1.0)
            eneg = np_.tile([P, ROW], F32, tag="eneg")
            nc.scalar.activation(eneg[:], rn2[:], AF.Exp, scale=-1.0)
            rp = np_.tile([P, ROW], F32, tag="rp")
            nc.scalar.activation(rp[:], zb[:], AF.Relu)
            helu = np_.tile([P, ROW], F32, tag="helu")
            nc.vector.scalar_tensor_tensor(helu[:], eneg[:], -1.0, rp[:],
                                           op0=OP.add, op1=OP.add)
            if final:
                hm = np_.tile([P, ROW], F32, tag="hm")
                nc.vector.tensor_tensor(out=hm[:], in0=helu[:],
                                        in1=pm_t[:, b:b + 1].to_broadcast([P, ROW]),
                                        op=OP.add)
                gidx = int(st.blk2graph[b])
                for j in range(2):
                    pt = pst.tile([P, P], F32, space="PSUM", tag="pt")
                    nc.tensor.transpose(out=pt[:], in_=hm[:, j * P:(j + 1) * P],
                                        identity=ident[:])
                    rmax = mp.tile([P, 1], F32, tag="rmax")
                    nc.vector.tensor_reduce(rmax[:], pt[:], axis=mybir.AxisListType.X,
                                            op=OP.max)
                    col = acc[:, j * st.Gmax + gidx: j * st.Gmax + gidx + 1]
                    nc.vector.tensor_tensor(out=col, in0=col, in1=rmax[:], op=OP.max)
            else:
                ps2 = psh.tile([P, AW], F32, space="PSUM", tag="ps2")
                for j in range(2):
                    pt = pst.tile([P, P], F32, space="PSUM", tag="pt")
                    nc.tensor.transpose(out=pt[:], in_=helu[:, j * P:(j + 1) * P],
                                        identity=ident[:])
                    lhs = mp.tile([P, P], F32, tag="lhs")
                    nc.vector.tensor_copy(lhs[:], pt[:])
                    nc.tensor.matmul(out=ps2[:], lhsT=lhs[:], rhs=w2t[j][:],
                                     start=(j == 0), stop=(j == 1))
                orow = np_.tile([P, AW], F32, tag="orow")
                nc.vector.tensor_copy(orow[:], ps2[:])
                nc.sync.dma_start(out[b * P:(b + 1) * P, :], orow[:])

        if final:
            # MLP: z1 = pooled @ lin1_w + b ; z2 = z1 @ lin2_w + b
            Gm = st.Gmax
            z1p = psh.tile([P, HC // 2], F32, space="PSUM", tag="ps2")
            for j in range(2):
                nc.tensor.matmul(out=z1p[:Gm, :],
                                 lhsT=acc[:, j * Gm:(j + 1) * Gm],
                                 rhs=l1w[j][:], start=(j == 0), stop=(j == 1))
            z1s = np_.tile([P, HC // 2], F32, tag="z1s")
            nc.vector.tensor_tensor(out=z1s[:Gm, :], in0=z1p[:Gm, :],
                                    in1=l1b[:Gm, :], op=OP.add)
            z1tp = pst.tile([P, P], F32, space="PSUM", tag="pt")
            nc.tensor.transpose(out=z1tp[:, :Gm], in_=z1s[:Gm, :],
                                identity=ident[:Gm, :Gm])
            z1T = np_.tile([P, P], F32, tag="z1T")
            nc.vector.tensor_copy(z1T[:, :Gm], z1tp[:, :Gm])
            z2p = psh.tile([P, 2], F32, space="PSUM", tag="ps2")
            nc.tensor.matmul(out=z2p[:Gm, :], lhsT=z1T[:, :Gm],
                             rhs=l2w[:HC // 2, :], start=True, stop=True)
            z2s = np_.tile([P, 2], F32, tag="z2s")
            nc.vector.tensor_tensor(out=z2s[:st.Gmax, :], in0=z2p[:st.Gmax, :],
                                    in1=l2b[:st.Gmax, :], op=OP.add)
            nc.sync.dma_start(out[:st.Gmax, :], z2s[:st.Gmax, :])
    nc.compile()
    return nc


class _PjrtRunner:
    """jit-once SPMD runner (mirrors bass2jax.run_bass_via_pjrt, cached)."""

    def __init__(self, nc, n_cores=NCORE):
        import jax
        from jax.sharding import Mesh, PartitionSpec
        from jax.experimental.shard_map import shard_map
        import concourse.mybir as mybir
        from concourse import bass2jax
        bass2jax.install_neuronx_cc_hook()
        self.nc = nc
        partition_name = (nc.partition_id_tensor.name
                          if nc.partition_id_tensor else None)
        in_names, out_names, out_avals = [], [], []
        for alloc in nc.m.functions[0].allocations:
            if not isinstance(alloc, mybir.MemoryLocationSet):
                continue
            name = alloc.memorylocations[0].name
            if alloc.kind == "ExternalInput":
                if name != partition_name:
                    in_names.append(name)
            elif alloc.kind == "ExternalOutput":
                out_names.append(name)
                out_avals.append(jax.core.ShapedArray(
                    tuple(alloc.tensor_shape), mybir.dt.np(alloc.dtype)))
        self.in_names, self.out_names, self.out_avals = in_names, out_names, out_avals
        self.n_cores = n_cores
        n_params = len(in_names)
        n_outs = len(out_names)
        all_names = in_names + out_names
        if partition_name is not None:
            all_names = all_names + [partition_name]

        def _body(*args):
            operands = list(args)
            if partition_name is not None:
                operands.append(bass2jax.partition_id_tensor())
            outs = bass2jax._bass_exec_p.bind(
                *operands,
                out_avals=tuple(out_avals),
                in_names=tuple(all_names),
                out_names=tuple(out_names),
                lowering_input_output_aliases=(),
                sim_require_finite=True,
                sim_require_nnan=True,
                nc=nc,
            )
            return tuple(outs)

        devices = jax.devices()[:n_cores]
        mesh = Mesh(np.asarray(devices), ("core",))
        donate = tuple(range(n_params, n_params + n_outs))
        self.sharded = jax.jit(
            shard_map(_body, mesh=mesh,
                      in_specs=(PartitionSpec("core"),) * (n_params + n_outs),
                      out_specs=(PartitionSpec("core"),) * n_outs,
                      check_rep=False),
            donate_argnums=donate, keep_unused=True)

    def concat_inputs(self, in_maps):
        return [np.concatenate([np.asarray(in_maps[c][n]) for c in range(self.n_cores)],
                               axis=0) for n in self.in_names]

    def fresh_zeros(self):
        return [np.zeros((self.n_cores * a.shape[0], *a.shape[1:]), a.dtype)
                for a in self.out_avals]

    def execute(self, concat_in, zeros):
        import jax
        out = self.sharded(*concat_in, *zeros)
        return jax.block_until_ready(out)

    def __call__(self, in_maps):
        out_arrs = self.execute(self.concat_inputs(in_maps), self.fresh_zeros())
        return [
            {n: np.asarray(out_arrs[i]).reshape(self.n_cores, *self.out_avals[i].shape)[c]
             for i, n in enumerate(self.out_names)}
            for c in range(self.n_cores)
        ]


def get_runners(st):
    global _DEV_CACHE
    key = (st.g2.T, st.g2.B, st.g3.T, st.B3, st.Gmax)
    if key not in _DEV_CACHE:
        _DEV_CACHE[key] = tuple(_PjrtRunner(nc) for nc in (
            _build_p1(st.g2.B),
            _build_edge_program(st.g2, st, final=False),
            _build_edge_program(st.g3, st, final=True),
        ))
    return _DEV_CACHE[key]


def _run_trn(st, inputs, trace=False):
    nc1, nc2, nc3 = get_runners(st)

    x = np.asarray(inputs['x'], np.float32)
    w1c = _combine_w(inputs['W1'], inputs['att_src1'], inputs['att_dst1'])
    w2c = _combine_w(inputs['W2'], inputs['att_src2'], inputs['att_dst2'])
    b1b = np.broadcast_to(np.asarray(inputs['b1'], np.float32), (P, ROW)).copy()
    b2b = np.broadcast_to(np.asarray(inputs['b2'], np.float32), (P, ROW)).copy()
    prof = {"exec_ns": [], "profiles": []}

    def run(runner, in_maps):
        return runner(in_maps)

    # ---- P1
    B = st.g2.B
    xT = np.ascontiguousarray(x.T)
    in1 = []
    for c in range(NCORE):
        sh = np.zeros((FIN, B * P), np.float32)
        n0, n1 = st.nb[c], st.nb[c + 1]
        sh[:, : n1 - n0] = xT[:, n0:n1]
        in1.append({"xT": sh, "w1c": w1c})
    r1 = run(nc1, in1)
    table1 = np.zeros((NN, ROW), np.float32)
    a1 = np.zeros((NN, 8), np.float32)
    for c in range(NCORE):
        n0, n1 = st.nb[c], st.nb[c + 1]
        o = r1[c]["out"]
        table1[n0:n1] = o[: n1 - n0, :ROW]
        a1[n0:n1] = o[: n1 - n0, ROW:]

    # ---- P2
    in2 = []
    for c in range(NCORE):
        in2.append({
            "table": table1,
            "idx16": st.g2.idx16[c],
            "dstloc": st.g2.dstloc[c],
            "uarr": _build_u(a1, st.g2.esg[c], st.g2.edg[c]),
            "biasb": b1b,
            "w2c": w2c,
        })
    r2 = run(nc2, in2)
    table2 = np.zeros((NN, ROW), np.float32)
    a2 = np.zeros((NN, 8), np.float32)
    for c in range(NCORE):
        n0, n1 = st.nb[c], st.nb[c + 1]
        o = r2[c]["out"]
        table2[n0:n1] = o[: n1 - n0, :ROW]
        a2[n0:n1] = o[: n1 - n0, ROW:]

    # ---- P3
    lin1_w = np.ascontiguousarray(np.asarray(inputs['lin1_w'], np.float32))
    lin2_w = np.ascontiguousarray(np.asarray(inputs['lin2_w'], np.float32))
    l1bb = np.broadcast_to(np.asarray(inputs['lin1_b'], np.float32), (P, HC // 2)).copy()
    l2bb = np.broadcast_to(np.asarray(inputs['lin2_b'], np.float32), (P, 2)).copy()
    in3 = []
    for c in range(NCORE):
        in3.append({
            "table": table2,
            "idx16": st.g3.idx16[c],
            "dstloc": st.g3.dstloc[c],
            "uarr": _build_u(a2, st.g3.esg[c], st.g3.edg[c]),
            "biasb": b2b,
            "pmask": st.poolmasks[c],
            "lin1w": lin1_w,
            "lin1bb": l1bb,
            "lin2w": lin2_w,
            "lin2bb": l2bb,
        })
    r3 = run(nc3, in3)
    z = np.zeros((GRAPHS, 2), np.float32)
    for c in range(NCORE):
        ncg = st.gb[c + 1] - st.gb[c]
        z[st.gb[c]:st.gb[c + 1]] = r3[c]["out"][:ncg]
    prof["runners"] = (nc1, nc2, nc3)
    prof["in_maps"] = (in1, in2, in3)
    return z, prof


_LAST_PROF = None


def kernel(**inputs):
    global _LAST_PROF
    st = _prep_structure(np.asarray(inputs['edge_index']), np.asarray(inputs['batch']))
    if os.environ.get("GAT_BACKEND", "trn") == "numpy":
        return _run_numpy(st, inputs)
    z, prof = _run_trn(st, inputs, trace=bool(os.environ.get("GAT_TRACE")))
    _LAST_PROF = prof
    return z


# revision 14
# speedup vs baseline: 2.1182x; 2.1182x over previous
"""Trainium2 Bass kernel for a 2-layer GAT + global-max-pool + MLP (GATNet2).

Strategy (8 NeuronCores, data-parallel over destination nodes / graphs):
  - Nodes are sharded across cores at graph boundaries (batch is sorted).
  - 3 SPMD launches:
      P1 "embed":  per-core  h1 = x @ [W1|wsrc1|wdst1]          -> table1, a1
      P2 "mid":    per-core  layer-1 edge phase (gather h1[src] via
                   dma_gather, attention-softmax-weighted segment-sum via
                   one-hot matmuls into PSUM), +bias, ELU, then @ W2c
                                                                 -> table2, a2
      P3 "final":  layer-2 edge phase, +bias, ELU, graph max-pool (PE
                   transpose + free-dim reduce over graph-aligned slot
                   blocks), 2-layer MLP                          -> logits
  - Host work between launches is indexing/assembly only: shard edges by
    dst, build int16 gather indices (lo/hi table split for the 32768 int16
    limit), gather per-edge attention pre-activations u = a_src[s]+a_dst[d]
    from the small device-computed a-tables.

Numerics: segment-softmax is computed without the segment-max subtraction
(logits are O(10), exp is safe in fp32); normalization is applied to the
accumulated sums instead of per-edge alphas. Both are mathematically
identical to the reference up to fp32 rounding.
"""
import os
import sys
import numpy as np

for _p in ("/opt/trn_rl_repo", "/root/.axon_site/_ro/trn_rl_repo"):
    if os.path.isdir(_p) and _p not in sys.path:
        sys.path.insert(0, _p)

# ---------------- problem constants (hardcoded per contest rules) ----------
NN = 50000        # nodes
EE = 400000       # edges (before self loops)
FIN = 256         # input features
HEADS = 4
CDIM = 64
HC = HEADS * CDIM  # 256
GRAPHS = 128
NCORE = 8
P = 128
LO = 32768        # int16 index limit -> lo/hi table split
NEG = 0.2         # leaky relu slope
ROW = 256         # gather row width (fp32 -> 1024B, %256==0)
AW = ROW + 8      # combined weight output width (h | a_src | a_dst)
PADLOC = 999.0    # dst-slot value for padding edges (matches no one-hot col)
MASKVAL = -1e30


BF16 = os.environ.get("GAT_F32") != "1"


def _table_np_dtype():
    if BF16:
        import ml_dtypes
        return ml_dtypes.bfloat16
    return np.float32


# ---------------- host-side grid / sharding ---------------------------------
class Grid:
    pass


def _core_bounds(batch):
    """Graph-aligned node ranges per core + graph ranges."""
    gcounts = np.bincount(batch, minlength=GRAPHS)
    cum = np.concatenate([[0], np.cumsum(gcounts)])  # [G+1] node offset per graph
    gb = [0]
    for c in range(1, NCORE):
        target = c * NN / NCORE
        gi = int(np.argmin(np.abs(cum - target)))
        gi = min(max(gi, gb[-1] + 1), GRAPHS - (NCORE - c))
        gb.append(gi)
    gb.append(GRAPHS)
    nb = [int(cum[g]) for g in gb]
    return np.array(nb), np.array(gb), gcounts, cum


def _build_edge_grid(s, d, nb, slot_of_local, nslots):
    """Static (block, half) tile schedule shared by all cores.

    s, d: global src/dst of all edges (self loops included)
    slot_of_local[c]: maps core-local node id -> slot id (identity for P2)
    nslots[c]: number of slots on core c
    Returns Grid with per-core packed arrays.
    """
    g = Grid()
    B = max(-(-int(n) // P) for n in nslots)
    g.B = B
    percore = []
    cnt = np.zeros((NCORE, B, 2), np.int64)
    for c in range(NCORE):
        m = (d >= nb[c]) & (d < nb[c + 1])
        es = s[m]
        ed = d[m]
        slot = slot_of_local[c][ed - nb[c]]
        blk = slot // P
        loc = slot % P
        half = (es >= LO).astype(np.int64)
        order = np.lexsort((half, blk))
        es, ed, blk, loc, half = es[order], ed[order], blk[order], loc[order], half[order]
        np.add.at(cnt[c], (blk, half), 1)
        percore.append((es, ed, blk, loc, half))
    K = -(-cnt.max(axis=0) // P)          # [B, 2] tiles per (block, half)
    g.K = K
    groups = []
    t0 = 0
    for b in range(B):
        for h in (0, 1):
            if K[b, h] > 0:
                groups.append((b, h, t0, int(K[b, h])))
                t0 += int(K[b, h])
    g.groups = groups
    g.T = t0

    g.idx16 = []
    g.dstloc = []
    g.esg = []
    g.edg = []
    g.oht = []
    ohdt = _table_np_dtype()
    for c in range(NCORE):
        es, ed, blk, loc, half = percore[c]
        # offsets of each (b, h) run in the sorted arrays
        off = np.zeros((B, 2), np.int64)
        run = np.cumsum(cnt[c].ravel())
        off.ravel()[1:] = run[:-1]
        idx16 = np.zeros((16, g.T * 8), np.int16)
        dstloc = np.full((P, g.T), PADLOC, np.float32)
        esg = np.zeros((P, g.T), np.int64)
        edg = np.zeros((P, g.T), np.int64)
        for (b, h, t0, Kbh) in groups:
            L = int(cnt[c, b, h])
            if L == 0:
                continue
            o = int(off[b, h])
            k = np.arange(L)
            tt = t0 + k // P
            pp = k % P
            dstloc[pp, tt] = loc[o:o + L]
            esg[pp, tt] = es[o:o + L]
            edg[pp, tt] = ed[o:o + L]
            idx16[k % 16, t0 * 8 + k // 16] = (es[o:o + L] - (LO if h else 0)).astype(np.int16)
        g.idx16.append(np.tile(idx16, (8, 1)))
        g.dstloc.append(dstloc)
        g.esg.append(esg)
        g.edg.append(edg)
        # host-built one-hot tiles [P, T*P]: oht[p, t*P + j] = (dstloc[p,t]==j)
        oh = (dstloc[:, :, None] == np.arange(P)[None, None, :])
        g.oht.append(np.ascontiguousarray(
            oh.reshape(P, g.T * P).astype(ohdt)))
    return g


def _prep_structure(edge_index, batch):
    """All input-data-dependent static structure (computed once)."""
    st = Grid()
    s0 = np.asarray(edge_index[0], np.int64)
    d0 = np.asarray(edge_index[1], np.int64)
    loop = np.arange(NN, dtype=np.int64)
    s = np.concatenate([s0, loop])
    d = np.concatenate([d0, loop])
    nb, gb, gcounts, cum = _core_bounds(np.asarray(batch, np.int64))
    st.nb, st.gb, st.gcounts, st.cum = nb, gb, gcounts, cum

    # ---- P2 grid: slots = local node ids
    ident = [np.arange(nb[c + 1] - nb[c], dtype=np.int64) for c in range(NCORE)]
    nslots = [nb[c + 1] - nb[c] for c in range(NCORE)]
    st.g2 = _build_edge_grid(s, d, nb, ident, nslots)

    # ---- P3 grid: graph-padded slots
    st.Gmax = max(int(gb[c + 1] - gb[c]) for c in range(NCORE))
    assert st.Gmax <= P
    # blocks per graph-position i: max over cores of ceil(nodes(c,i)/P)
    Pi = np.zeros(st.Gmax, np.int64)
    for c in range(NCORE):
        for i in range(gb[c + 1] - gb[c]):
            Pi[i] = max(Pi[i], -(-int(gcounts[gb[c] + i]) // P))
    Pi = np.maximum(Pi, 1)
    st.Pi = Pi
    blk_base = np.concatenate([[0], np.cumsum(Pi)])
    st.B3 = int(blk_base[-1])
    st.blk2graph = np.concatenate([np.full(int(Pi[i]), i, np.int64)
                                   for i in range(st.Gmax)])
    slotmaps = []
    poolmasks = []
    for c in range(NCORE):
        ncg = int(gb[c + 1] - gb[c])
        smap = np.zeros(nb[c + 1] - nb[c], np.int64)
        pmask = np.full((P, st.B3), MASKVAL, np.float32)
        for i in range(ncg):
            gs = int(cum[gb[c] + i]) - nb[c]       # local node start
            ge = int(cum[gb[c] + i + 1]) - nb[c]
            base = int(blk_base[i]) * P
            smap[gs:ge] = base + np.arange(ge - gs)
            for bb in range(int(Pi[i])):
                v0 = bb * P
                v1 = min(ge - gs, (bb + 1) * P)
                if v1 > v0:
                    pmask[: v1 - v0, int(blk_base[i]) + bb] = 0.0
        slotmaps.append(smap)
        poolmasks.append(pmask)
    st.poolmasks = poolmasks
    st.g3 = _build_edge_grid(s, d, nb, slotmaps, [st.B3 * P] * NCORE)
    return st


def _combine_w(W, att_src, att_dst):
    W = np.asarray(W, np.float64)
    ws = (W.reshape(FIN if W.shape[0] == FIN else HC, HEADS, CDIM)
          * np.asarray(att_src, np.float64)[None]).sum(-1)
    wd = (W.reshape(-1, HEADS, CDIM)
          * np.asarray(att_dst, np.float64)[None]).sum(-1)
    return np.concatenate([W, ws, wd], axis=1).astype(np.float32)  # [K, 264]


def _build_u(a_full, esg, edg):
    """Per-edge attention pre-activation u = a_src[s] + a_dst[d], packed."""
    a_src = a_full[:, 0:4]
    a_dst = a_full[:, 4:8]
    u = a_src[esg] + a_dst[edg]          # [P, T, 4]
    return np.ascontiguousarray(u.reshape(P, -1), dtype=np.float32)


# ---------------- numpy emulation of the device programs ---------------------
def _np_edge_phase(grid, table, u, c):
    """Emulate P2/P3 edge phase for core c -> per-block [P, 260] results."""
    ex = np.exp(np.where(u > 0, u, NEG * u)).reshape(P, grid.T, 4)
    dstloc = grid.dstloc[c]
    esg = grid.esg[c]
    out = np.zeros((grid.B, P, ROW + 4), np.float32)
    for (b, h, t0, K) in grid.groups:
        for k in range(K):
            t = t0 + k
            onehot = (dstloc[:, t][:, None] == np.arange(P)[None, :]).astype(np.float32)
            gath = table[esg[:, t]]                       # [P, 256]
            msg = np.empty((P, ROW + 4), np.float32)
            msg[:, :ROW] = (gath.reshape(P, 4, 64) * ex[:, t, :, None]).reshape(P, ROW)
            msg[:, ROW:] = ex[:, t]
            out[b] += onehot.T @ msg
    return out


def _np_norm_elu(blk, bias):
    den = blk[:, ROW:] + 1e-16
    go = (blk[:, :ROW].reshape(P, 4, 64) / den[:, :, None]).reshape(P, ROW)
    z = go + bias[None, :]
    return np.where(z > 0, z, np.exp(np.minimum(z, 0)) - 1).astype(np.float32)


def _run_numpy(st, inputs):
    x = np.asarray(inputs['x'], np.float32)
    w1c = _combine_w(inputs['W1'], inputs['att_src1'], inputs['att_dst1'])
    w2c = _combine_w(inputs['W2'], inputs['att_src2'], inputs['att_dst2'])
    b1 = np.asarray(inputs['b1'], np.float32)
    b2 = np.asarray(inputs['b2'], np.float32)
    # P1
    h1full = x @ w1c                       # [N, 264]
    table1 = np.ascontiguousarray(h1full[:, :ROW])
    a1 = h1full[:, ROW:]
    # P2
    table2 = np.zeros((NN, ROW), np.float32)
    a2 = np.zeros((NN, 8), np.float32)
    for c in range(NCORE):
        u = _build_u(a1, st.g2.esg[c], st.g2.edg[c])
        blks = _np_edge_phase(st.g2, table1, u, c)
        n0, n1 = st.nb[c], st.nb[c + 1]
        rows = []
        for b in range(st.g2.B):
            rows.append(_np_norm_elu(blks[b], b1))
        helu = np.concatenate(rows, 0)[: n1 - n0]
        h2 = helu @ w2c
        table2[n0:n1] = h2[:, :ROW]
        a2[n0:n1] = h2[:, ROW:]
    # P3
    z = np.zeros((GRAPHS, 2), np.float32)
    lin1_w = np.asarray(inputs['lin1_w'], np.float32)
    lin1_b = np.asarray(inputs['lin1_b'], np.float32)
    lin2_w = np.asarray(inputs['lin2_w'], np.float32)
    lin2_b = np.asarray(inputs['lin2_b'], np.float32)
    for c in range(NCORE):
        u = _build_u(a2, st.g3.esg[c], st.g3.edg[c])
        blks = _np_edge_phase(st.g3, table2, u, c)
        acc = np.full((st.Gmax, ROW), MASKVAL, np.float32)
        for b in range(st.B3):
            helu = _np_norm_elu(blks[b], b2)
            hm = helu + st.poolmasks[c][:, b][:, None]
            i = st.blk2graph[b]
            acc[i] = np.maximum(acc[i], hm.max(axis=0))
        ncg = st.gb[c + 1] - st.gb[c]
        z1 = acc[:ncg] @ lin1_w + lin1_b
        z2 = z1 @ lin2_w + lin2_b
        z[st.gb[c]:st.gb[c + 1]] = z2
    return z


# ---------------- Trainium programs ------------------------------------------
_DEV_CACHE = {}


def _get_bass_modules():
    import concourse.bass as bass
    import concourse.mybir as mybir
    import concourse.tile as tile
    import concourse.bacc as bacc
    from concourse import bass_utils
    from concourse.library_config import mlp
    from concourse.masks import make_identity
    return bass, mybir, tile, bacc, bass_utils, mlp, make_identity


def _build_p1(B):
    bass, mybir, tile, bacc, bass_utils, mlp, make_identity = _get_bass_modules()
    from contextlib import ExitStack
    F32 = mybir.dt.float32
    TD = mybir.dt.bfloat16 if BF16 else F32
    nc = bacc.Bacc("TRN2", target_bir_lowering=False, debug=False, num_devices=NCORE)
    xT = nc.dram_tensor("xT", [FIN, B * P], F32, kind="ExternalInput").ap()
    w1c = nc.dram_tensor("w1c", [FIN, AW], F32, kind="ExternalInput").ap()
    outh = nc.dram_tensor("outh", [B * P, ROW], TD, kind="ExternalOutput").ap()
    outa = nc.dram_tensor("outa", [B * P, 8], F32, kind="ExternalOutput").ap()
    with tile.TileContext(nc) as tc, ExitStack() as ctx:
        sb = ctx.enter_context(tc.tile_pool(name="sb", bufs=3))
        wp = ctx.enter_context(tc.tile_pool(name="wp", bufs=1))
        ps = ctx.enter_context(tc.tile_pool(name="ps", bufs=2, space="PSUM"))
        wt = []
        xt = []
        for j in range(2):
            w = wp.tile([P, AW], F32, tag=f"w{j}")
            nc.sync.dma_start(w[:], w1c[j * P:(j + 1) * P, :])
            wt.append(w)
            xbig = wp.tile([P, B * P], F32, tag=f"x{j}")
            nc.sync.dma_start(xbig[:], xT[j * P:(j + 1) * P, :])
            xt.append(xbig)
        for b in range(B):
            pso = ps.tile([P, AW], F32, space="PSUM", tag="pso")
            for j in range(2):
                nc.tensor.matmul(out=pso[:], lhsT=xt[j][:, b * P:(b + 1) * P],
                                 rhs=wt[j][:], start=(j == 0), stop=(j == 1))
            oh_t = sb.tile([P, ROW], TD, tag="oh_t")
            nc.vector.tensor_copy(oh_t[:], pso[:, :ROW])
            oa_t = sb.tile([P, 8], F32, tag="oa_t")
            nc.vector.tensor_copy(oa_t[:], pso[:, ROW:])
            nc.sync.dma_start(outh[b * P:(b + 1) * P, :], oh_t[:])
            nc.sync.dma_start(outa[b * P:(b + 1) * P, :], oa_t[:])
    nc.compile()
    return nc


def _build_edge_program(grid, st, final):
    bass, mybir, tile, bacc, bass_utils, mlp, make_identity = _get_bass_modules()
    from contextlib import ExitStack
    F32 = mybir.dt.float32
    I16 = mybir.dt.int16
    I32 = mybir.dt.int32
    AF = mybir.ActivationFunctionType
    OP = mybir.AluOpType
    T, B = grid.T, (st.B3 if final else grid.B)

    TD = mybir.dt.bfloat16 if BF16 else F32
    nc = bacc.Bacc("TRN2", target_bir_lowering=False, debug=False, num_devices=NCORE)
    table = nc.dram_tensor("table", [NN, ROW], TD, kind="ExternalInput").ap()
    idx16 = nc.dram_tensor("idx16", [P, T * 8], I16, kind="ExternalInput").ap()
    ohtd = nc.dram_tensor("ohtd", [P, T * P], TD, kind="ExternalInput").ap()
    uarr = nc.dram_tensor("uarr", [P, T * 4], F32, kind="ExternalInput").ap()
    biasb = nc.dram_tensor("biasb", [P, ROW], F32, kind="ExternalInput").ap()
    if final:
        pmask = nc.dram_tensor("pmask", [P, B], F32, kind="ExternalInput").ap()
        lin1w = nc.dram_tensor("lin1w", [HC, HC // 2], F32, kind="ExternalInput").ap()
        lin1bb = nc.dram_tensor("lin1bb", [P, HC // 2], F32, kind="ExternalInput").ap()
        lin2w = nc.dram_tensor("lin2w", [HC // 2, 2], F32, kind="ExternalInput").ap()
        lin2bb = nc.dram_tensor("lin2bb", [P, 2], F32, kind="ExternalInput").ap()
        out = nc.dram_tensor("out", [P, 2], F32, kind="ExternalOutput").ap()
    else:
        w2c = nc.dram_tensor("w2c", [HC, AW], F32, kind="ExternalInput").ap()
        outh = nc.dram_tensor("outh", [B * P, ROW], TD, kind="ExternalOutput").ap()
        outa = nc.dram_tensor("outa", [B * P, 8], F32, kind="ExternalOutput").ap()

    with tile.TileContext(nc) as tc, ExitStack() as ctx:
        cons = ctx.enter_context(tc.tile_pool(name="cons", bufs=1))
        meta = ctx.enter_context(tc.tile_pool(name="meta", bufs=1))
        gp = ctx.enter_context(tc.tile_pool(name="gp", bufs=3))
        mp = ctx.enter_context(tc.tile_pool(name="mp", bufs=4))
        ohp = ctx.enter_context(tc.tile_pool(name="ohp", bufs=4))
        np_ = ctx.enter_context(tc.tile_pool(name="np", bufs=2))
        ps = ctx.enter_context(tc.tile_pool(name="ps", bufs=2, space="PSUM"))
        pst = ctx.enter_context(tc.tile_pool(name="pst", bufs=2, space="PSUM"))
        psh = ctx.enter_context(tc.tile_pool(name="psh", bufs=1, space="PSUM"))

        nc.gpsimd.load_library(mlp)

        # constants
        ident = cons.tile([P, P], F32)
        make_identity(nc, ident[:])
        bias_t = cons.tile([P, ROW], F32)
        nc.sync.dma_start(bias_t[:], biasb[:, :])

        # metadata loads
        idx_t = meta.tile([P, T * 8], I16)
        nc.sync.dma_start(idx_t[:], idx16[:, :])
        ex_t = meta.tile([P, T * 4], TD)

        # ex pre-pass: ex = exp(leaky_relu(u)); lrelu(u) = u + 0.8*relu(-u)
        # (lrelu kept in f32; only the exp output is quantized to TD)
        CH = 2048
        u_t = meta.tile([P, T * 4], F32)
        nc.sync.dma_start(u_t[:], uarr[:, :])
        for o in range(0, T * 4, CH):
            e = min(o + CH, T * 4)
            rn = mp.tile([P, CH], F32, tag="rn")
            nc.scalar.activation(rn[:, :e - o], u_t[:, o:e], AF.Relu, scale=-1.0)
            lr = mp.tile([P, CH], F32, tag="lr")
            nc.vector.scalar_tensor_tensor(lr[:, :e - o], rn[:, :e - o], 0.8,
                                           u_t[:, o:e], op0=OP.mult, op1=OP.add)
            nc.scalar.activation(ex_t[:, o:e], lr[:, :e - o], AF.Exp)

        if final:
            pm_t = meta.tile([P, B], F32)
            nc.sync.dma_start(pm_t[:], pmask[:, :])
            acc = cons.tile([P, 2 * st.Gmax], F32)
            nc.vector.memset(acc[:], MASKVAL)
            l1w = []
            for j in range(2):
                w = cons.tile([P, HC // 2], F32, tag=f"l1w{j}")
                nc.sync.dma_start(w[:], lin1w[j * P:(j + 1) * P, :])
                l1w.append(w)
            l1b = cons.tile([P, HC // 2], F32)
            nc.sync.dma_start(l1b[:], lin1bb[:, :])
            l2w = cons.tile([P, 2], F32)
            nc.sync.dma_start(l2w[:], lin2w[:, :])
            l2b = cons.tile([P, 2], F32)
            nc.sync.dma_start(l2b[:], lin2bb[:, :])
        else:
            w2t = []
            for j in range(2):
                w = cons.tile([P, AW], F32, tag=f"w2{j}")
                nc.sync.dma_start(w[:], w2c[j * P:(j + 1) * P, :])
                w2t.append(w)

        # group schedule per block
        by_block = [[] for _ in range(B)]
        for (b, h, t0, K) in grid.groups:
            by_block[b].append((h, t0, K))

        for b in range(B):
            glist = by_block[b]
            pso = ps.tile([P, ROW + 4], F32, space="PSUM", tag="pso")
            nt = sum(K for (_h, _t0, K) in glist)
            done = 0
            if nt == 0:
                continue
            for (h, t0, K) in glist:
                gb_t = gp.tile([P, K * ROW], F32, tag="gath")
                src_ap = table[:LO, :] if h == 0 else table[LO:, :]
                nc.gpsimd.dma_gather(
                    out_ap=gb_t[:].rearrange("p (k d) -> p k d", d=ROW),
                    in_ap=src_ap,
                    idxs_ap=idx_t[:, t0 * 8:(t0 + K) * 8],
                    num_idxs=K * P,
                    num_idxs_reg=K * P,
                    elem_size=ROW,
                )
                for k in range(K):
                    t = t0 + k
                    oh = ohp.tile([P, P], F32, tag="oh")
                    nc.vector.tensor_tensor(
                        out=oh[:], in0=dl_t[:, t:t + 1].to_broadcast([P, P]),
                        in1=iota_f[:], op=OP.is_equal)
                    msg = mp.tile([P, ROW + 4], F32, tag="msg")
                    nc.vector.tensor_tensor(
                        out=msg[:, :ROW].rearrange("p (h c) -> p h c", c=CDIM),
                        in0=gb_t[:, k * ROW:(k + 1) * ROW].rearrange(
                            "p (h c) -> p h c", c=CDIM),
                        in1=ex_t[:, t * 4:(t + 1) * 4].to_broadcast([P, 4, CDIM]),
                        op=OP.mult)
                    nc.vector.tensor_copy(msg[:, ROW:], ex_t[:, t * 4:(t + 1) * 4])
                    nc.tensor.matmul(out=pso[:], lhsT=oh[:], rhs=msg[:],
                                     start=(done == 0), stop=(done == nt - 1))
                    done += 1
            # normalize + bias + ELU
            den = np_.tile([P, 4], F32, tag="den")
            nc.vector.tensor_scalar_add(den[:], pso[:, ROW:], 1e-16)
            rec = np_.tile([P, 4], F32, tag="rec")
            nc.vector.reciprocal(rec[:], den[:])
            zb = np_.tile([P, ROW], F32, tag="zb")
            nc.vector.tensor_tensor(
                out=zb[:].rearrange("p (h c) -> p h c", c=CDIM),
                in0=pso[:, :ROW].rearrange("p (h c) -> p h c", c=CDIM),
                in1=rec[:].to_broadcast([P, 4, CDIM]), op=OP.mult)
            nc.vector.tensor_tensor(out=zb[:], in0=zb[:], in1=bias_t[:], op=OP.add)
            # elu: rp + exp(min(z,0)) - 1
            rn2 = np_.tile([P, ROW], F32, tag="rn2")
            nc.scalar.activation(rn2[:], zb[:], AF.Relu, scale=-1.0)
            eneg = np_.tile([P, ROW], F32, tag="eneg")
            nc.scalar.activation(eneg[:], rn2[:], AF.Exp, scale=-1.0)
            rp = np_.tile([P, ROW], F32, tag="rp")
            nc.scalar.activation(rp[:], zb[:], AF.Relu)
            helu = np_.tile([P, ROW], F32, tag="helu")
            nc.vector.scalar_tensor_tensor(helu[:], eneg[:], -1.0, rp[:],
                                           op0=OP.add, op1=OP.add)
            if final:
                hm = np_.tile([P, ROW], F32, tag="hm")
                nc.vector.tensor_tensor(out=hm[:], in0=helu[:],
                                        in1=pm_t[:, b:b + 1].to_broadcast([P, ROW]),
                                        op=OP.add)
                gidx = int(st.blk2graph[b])
                for j in range(2):
                    pt = pst.tile([P, P], F32, space="PSUM", tag="pt")
                    nc.tensor.transpose(out=pt[:], in_=hm[:, j * P:(j + 1) * P],
                                        identity=ident[:])
                    rmax = mp.tile([P, 1], F32, tag="rmax")
                    nc.vector.tensor_reduce(rmax[:], pt[:], axis=mybir.AxisListType.X,
                                            op=OP.max)
                    col = acc[:, j * st.Gmax + gidx: j * st.Gmax + gidx + 1]
                    nc.vector.tensor_tensor(out=col, in0=col, in1=rmax[:], op=OP.max)
            else:
                ps2 = psh.tile([P, AW], F32, space="PSUM", tag="ps2")
                for j in range(2):
                    pt = pst.tile([P, P], F32, space="PSUM", tag="pt")
                    nc.tensor.transpose(out=pt[:], in_=helu[:, j * P:(j + 1) * P],
                                        identity=ident[:])
                    lhs = mp.tile([P, P], F32, tag="lhs")
                    nc.vector.tensor_copy(lhs[:], pt[:])
                    nc.tensor.matmul(out=ps2[:], lhsT=lhs[:], rhs=w2t[j][:],
                                     start=(j == 0), stop=(j == 1))
                orow = np_.tile([P, AW], F32, tag="orow")
                nc.vector.tensor_copy(orow[:], ps2[:])
                nc.sync.dma_start(out[b * P:(b + 1) * P, :], orow[:])

        if final:
            # MLP: z1 = pooled @ lin1_w + b ; z2 = z1 @ lin2_w + b
            Gm = st.Gmax
            z1p = psh.tile([P, HC // 2], F32, space="PSUM", tag="ps2")
            for j in range(2):
                nc.tensor.matmul(out=z1p[:Gm, :],
                                 lhsT=acc[:, j * Gm:(j + 1) * Gm],
                                 rhs=l1w[j][:], start=(j == 0), stop=(j == 1))
            z1s = np_.tile([P, HC // 2], F32, tag="z1s")
            nc.vector.tensor_tensor(out=z1s[:Gm, :], in0=z1p[:Gm, :],
                                    in1=l1b[:Gm, :], op=OP.add)
            z1tp = pst.tile([P, P], F32, space="PSUM", tag="pt")
            nc.tensor.transpose(out=z1tp[:, :Gm], in_=z1s[:Gm, :],
                                identity=ident[:Gm, :Gm])
            z1T = np_.tile([P, P], F32, tag="z1T")
            nc.vector.tensor_copy(z1T[:, :Gm], z1tp[:, :Gm])
            z2p = psh.tile([P, 2], F32, space="PSUM", tag="ps2")
            nc.tensor.matmul(out=z2p[:Gm, :], lhsT=z1T[:, :Gm],
                             rhs=l2w[:HC // 2, :], start=True, stop=True)
            z2s = np_.tile([P, 2], F32, tag="z2s")
            nc.vector.tensor_tensor(out=z2s[:st.Gmax, :], in0=z2p[:st.Gmax, :],
                                    in1=l2b[:st.Gmax, :], op=OP.add)
            nc.sync.dma_start(out[:st.Gmax, :], z2s[:st.Gmax, :])
    nc.compile()
    return nc


class _PjrtRunner:
    """jit-once SPMD runner (mirrors bass2jax.run_bass_via_pjrt, cached)."""

    def __init__(self, nc, n_cores=NCORE):
        import jax
        from jax.sharding import Mesh, PartitionSpec
        from jax.experimental.shard_map import shard_map
        import concourse.mybir as mybir
        from concourse import bass2jax
        bass2jax.install_neuronx_cc_hook()
        self.nc = nc
        partition_name = (nc.partition_id_tensor.name
                          if nc.partition_id_tensor else None)
        in_names, out_names, out_avals = [], [], []
        for alloc in nc.m.functions[0].allocations:
            if not isinstance(alloc, mybir.MemoryLocationSet):
                continue
            name = alloc.memorylocations[0].name
            if alloc.kind == "ExternalInput":
                if name != partition_name:
                    in_names.append(name)
            elif alloc.kind == "ExternalOutput":
                out_names.append(name)
                out_avals.append(jax.core.ShapedArray(
                    tuple(alloc.tensor_shape), mybir.dt.np(alloc.dtype)))
        self.in_names, self.out_names, self.out_avals = in_names, out_names, out_avals
        self.n_cores = n_cores
        n_params = len(in_names)
        n_outs = len(out_names)
        all_names = in_names + out_names
        if partition_name is not None:
            all_names = all_names + [partition_name]

        def _body(*args):
            operands = list(args)
            if partition_name is not None:
                operands.append(bass2jax.partition_id_tensor())
            outs = bass2jax._bass_exec_p.bind(
                *operands,
                out_avals=tuple(out_avals),
                in_names=tuple(all_names),
                out_names=tuple(out_names),
                lowering_input_output_aliases=(),
                sim_require_finite=True,
                sim_require_nnan=True,
                nc=nc,
            )
            return tuple(outs)

        devices = jax.devices()[:n_cores]
        mesh = Mesh(np.asarray(devices), ("core",))
        self.mesh = mesh
        self.pspec = PartitionSpec("core")
        donate = tuple(range(n_params, n_params + n_outs))
        self.sharded = jax.jit(
            shard_map(_body, mesh=mesh,
                      in_specs=(PartitionSpec("core"),) * (n_params + n_outs),
                      out_specs=(PartitionSpec("core"),) * n_outs,
                      check_rep=False),
            donate_argnums=donate, keep_unused=True)

    def concat_inputs(self, in_maps):
        return [np.concatenate([np.asarray(in_maps[c][n]) for c in range(self.n_cores)],
                               axis=0) for n in self.in_names]

    def fresh_zeros(self):
        return [np.zeros((self.n_cores * a.shape[0], *a.shape[1:]), a.dtype)
                for a in self.out_avals]

    def shard_put(self, arrs):
        import jax
        from jax.sharding import NamedSharding
        sh = NamedSharding(self.mesh, self.pspec)
        out = [jax.device_put(a, sh) for a in arrs]
        jax.block_until_ready(out)
        return out

    def execute(self, concat_in, zeros):
        import jax
        out = self.sharded(*concat_in, *zeros)
        return jax.block_until_ready(out)

    def __call__(self, in_maps):
        out_arrs = self.execute(self.concat_inputs(in_maps), self.fresh_zeros())
        return [
            {n: np.asarray(out_arrs[i]).reshape(self.n_cores, *self.out_avals[i].shape)[c]
             for i, n in enumerate(self.out_names)}
            for c in range(self.n_cores)
        ]


def get_runners(st):
    global _DEV_CACHE
    key = (st.g2.T, st.g2.B, st.g3.T, st.B3, st.Gmax)
    if key not in _DEV_CACHE:
        _DEV_CACHE[key] = tuple(_PjrtRunner(nc) for nc in (
            _build_p1(st.g2.B),
            _build_edge_program(st.g2, st, final=False),
            _build_edge_program(st.g3, st, final=True),
        ))
    return _DEV_CACHE[key]


def _run_trn(st, inputs, trace=False):
    nc1, nc2, nc3 = get_runners(st)

    x = np.asarray(inputs['x'], np.float32)
    w1c = _combine_w(inputs['W1'], inputs['att_src1'], inputs['att_dst1'])
    w2c = _combine_w(inputs['W2'], inputs['att_src2'], inputs['att_dst2'])
    b1b = np.broadcast_to(np.asarray(inputs['b1'], np.float32), (P, ROW)).copy()
    b2b = np.broadcast_to(np.asarray(inputs['b2'], np.float32), (P, ROW)).copy()
    prof = {"exec_ns": [], "profiles": []}

    def run(runner, in_maps):
        return runner(in_maps)

    # ---- P1
    B = st.g2.B
    xT = np.ascontiguousarray(x.T)
    in1 = []
    for c in range(NCORE):
        sh = np.zeros((FIN, B * P), np.float32)
        n0, n1 = st.nb[c], st.nb[c + 1]
        sh[:, : n1 - n0] = xT[:, n0:n1]
        in1.append({"xT": sh, "w1c": w1c})
    r1 = run(nc1, in1)
    table1 = np.zeros((NN, ROW), np.float32)
    a1 = np.zeros((NN, 8), np.float32)
    for c in range(NCORE):
        n0, n1 = st.nb[c], st.nb[c + 1]
        o = r1[c]["out"]
        table1[n0:n1] = o[: n1 - n0, :ROW]
        a1[n0:n1] = o[: n1 - n0, ROW:]

    # ---- P2
    in2 = []
    for c in range(NCORE):
        in2.append({
            "table": table1,
            "idx16": st.g2.idx16[c],
            "dstloc": st.g2.dstloc[c],
            "uarr": _build_u(a1, st.g2.esg[c], st.g2.edg[c]),
            "biasb": b1b,
            "w2c": w2c,
        })
    r2 = run(nc2, in2)
    table2 = np.zeros((NN, ROW), np.float32)
    a2 = np.zeros((NN, 8), np.float32)
    for c in range(NCORE):
        n0, n1 = st.nb[c], st.nb[c + 1]
        o = r2[c]["out"]
        table2[n0:n1] = o[: n1 - n0, :ROW]
        a2[n0:n1] = o[: n1 - n0, ROW:]

    # ---- P3
    lin1_w = np.ascontiguousarray(np.asarray(inputs['lin1_w'], np.float32))
    lin2_w = np.ascontiguousarray(np.asarray(inputs['lin2_w'], np.float32))
    l1bb = np.broadcast_to(np.asarray(inputs['lin1_b'], np.float32), (P, HC // 2)).copy()
    l2bb = np.broadcast_to(np.asarray(inputs['lin2_b'], np.float32), (P, 2)).copy()
    in3 = []
    for c in range(NCORE):
        in3.append({
            "table": table2,
            "idx16": st.g3.idx16[c],
            "dstloc": st.g3.dstloc[c],
            "uarr": _build_u(a2, st.g3.esg[c], st.g3.edg[c]),
            "biasb": b2b,
            "pmask": st.poolmasks[c],
            "lin1w": lin1_w,
            "lin1bb": l1bb,
            "lin2w": lin2_w,
            "lin2bb": l2bb,
        })
    r3 = run(nc3, in3)
    z = np.zeros((GRAPHS, 2), np.float32)
    for c in range(NCORE):
        ncg = st.gb[c + 1] - st.gb[c]
        z[st.gb[c]:st.gb[c + 1]] = r3[c]["out"][:ncg]
    prof["runners"] = (nc1, nc2, nc3)
    prof["in_maps"] = (in1, in2, in3)
    return z, prof


_LAST_PROF = None


def kernel(**inputs):
    global _LAST_PROF
    st = _prep_structure(np.asarray(inputs['edge_index']), np.asarray(inputs['batch']))
    if os.environ.get("GAT_BACKEND", "trn") == "numpy":
        return _run_numpy(st, inputs)
    z, prof = _run_trn(st, inputs, trace=bool(os.environ.get("GAT_TRACE")))
    _LAST_PROF = prof
    return z
